# revision 16
# baseline (speedup 1.0000x reference)
"""Trainium2 Bass kernel for nn_LocalStyleAdaptor (WaveNet-ish encoder + VQ codebook).

Data parallel over 8 NeuronCores: batch 16 -> 2 per core. Full conv stack in
fp32 (VQ argmin is sensitive to sub-1e-5 perturbations of the encoder output;
bf16 convs flip ~0.5% of code assignments -> ~10% z error). Channels live on
partitions, time on the free dim. WN/post convs run per-tap (shifted rhs
column windows, no copies); the heavy encoder conv1 (k=5) K-packs taps padded
to 96 rows so every stack copy is 32-partition aligned. LayerNorm runs in a
transposed (position-on-partition) sandwich via PE transposes. VQ computes
full distances (||x||^2 + ||e||^2 - 2 x.e) on the PE array exactly like the
reference (same fp32 tie rounding), argmin via reverse-iota reduce, q via
onehot-transpose matmul (bit-exact codebook rows).
"""

import numpy as np

C = 80
H = 256
M = 64
T = 4000
B_SH = 2          # batch per core
N_CORES = 8
TILE = 500        # time-tile width
NT = T // TILE    # tiles per batch
PCH = 125         # VQ position chunk
PAD = 2           # halo pad columns on each side of resident buffers
TB = T + 2 * PAD  # padded buffer width
SCALE5 = float(5.0 ** -0.5)

_CACHE = {}
LIMIT_PHASES = 99  # for profiling: 0=transpose only,1=+WN,2=+enc,3=+post/VQ


def _to_np(x):
    return np.asarray(x, dtype=np.float32)


def _prep_weights(params):
    p = {k: ([_to_np(v) for v in params[k]] if isinstance(params[k], (list, tuple))
             else _to_np(params[k])) for k in params}
    w = {}

    # WN input convs, per-tap lhsT [80, 160]
    wn_tap = np.zeros((4, 3, 80, 160), np.float32)
    wn_in_bT = np.zeros((80, 4, 2), np.float32)
    wn_rs = np.zeros((4, 80, 160), np.float32)
    wn_rs_bT = np.zeros((80, 4, 2), np.float32)
    for l in range(4):
        for k in range(3):
            wn_tap[l, k] = p['wn_in_w'][l][:, :, k].T
        wn_in_bT[:, l, 0] = p['wn_in_b'][l][0:80]
        wn_in_bT[:, l, 1] = p['wn_in_b'][l][80:160]
        rw = p['wn_rs_w'][l][:, :, 0].T            # [80, Cout]
        wn_rs[l, :, :rw.shape[1]] = rw
        rb = p['wn_rs_b'][l]
        wn_rs_bT[:, l, 0] = rb[0:80]
        if l < 3:
            wn_rs_bT[:, l, 1] = rb[80:160]
    w['wn_tap'] = np.ascontiguousarray(wn_tap.transpose(2, 0, 1, 3))  # [80,4,3,160]
    w['wn_rs'] = np.ascontiguousarray(wn_rs.transpose(1, 0, 2))        # [80,4,160]
    w['wn_in_bT'], w['wn_rs_bT'] = wn_in_bT, wn_rs_bT

    # encoder conv1: taps padded to 96 rows -> stack of 480 rows, 4 chunks
    e1 = np.zeros((10, 3, 128, 160), np.float32)
    e1d = np.zeros((10, 96, 160), np.float32)
    e_b1T = np.zeros((80, 10, 2), np.float32)
    e2w = np.zeros((10, 2, 80, 80), np.float32)
    e_b2T = np.zeros((80, 10), np.float32)
    for l in range(10):
        w1 = p['enc_w1'][l].copy()                 # [160, 80, 5]
        b1 = p['enc_b1'][l].copy()
        lnw, lnb = p['enc_ln_w'][l], p['enc_ln_b'][l]
        if not (np.all(lnw == 1.0) and np.all(lnb == 0.0)):
            b1 = b1 + np.einsum('ock,c->o', w1, lnb)
            w1 = w1 * lnw[None, :, None]
        st = np.zeros((480, 160), np.float32)
        for k in range(5):
            st[96 * k:96 * k + 80] = w1[:, :, k].T
        e1[l, 0], e1[l, 1], e1[l, 2] = st[0:128], st[128:256], st[256:384]
        e1d[l] = st[384:480]
        b1s = b1 * SCALE5
        e_b1T[:, l, 0] = b1s[0:80]
        e_b1T[:, l, 1] = b1s[80:160]
        w2 = p['enc_w2'][l][:, :, 0].T             # [160, 80]
        e2w[l, 0] = w2[0:80]
        e2w[l, 1] = w2[80:160]
        e_b2T[:, l] = p['enc_b2'][l]
    w['e1'] = np.ascontiguousarray(e1.transpose(2, 0, 1, 3))    # [128,10,3,160]
    w['e1d'] = np.ascontiguousarray(e1d.transpose(1, 0, 2))      # [96,10,160]
    w['e2w'] = np.ascontiguousarray(e2w.transpose(2, 0, 1, 3))   # [80,10,2,80]
    w['e_b1T'], w['e_b2T'] = e_b1T, e_b2T

    pw = p['post_w'].copy()                        # [256, 80, 3]
    pb = p['post_b'].copy()
    lnw, lnb = p['last_ln_w'], p['last_ln_b']
    if not (np.all(lnw == 1.0) and np.all(lnb == 0.0)):
        pb = pb + np.einsum('ock,c->o', pw, lnb)
        pw = pw * lnw[None, :, None]
    post_tap = np.zeros((3, 80, 256), np.float32)
    for k in range(3):
        post_tap[k] = pw[:, :, k].T
    w['post_tap'] = np.ascontiguousarray(post_tap.transpose(1, 0, 2))  # [80,3,256]
    post_bT = np.zeros((128, 2), np.float32)
    post_bT[:, 0] = pb[0:128]
    post_bT[:, 1] = pb[128:256]
    w['post_bT'] = post_bT

    emb = p['embedding']                           # [64, 256]
    embT = emb.T
    w['vq_rhs1'] = np.ascontiguousarray(-2.0 * embT[0:128])
    w['vq_rhs2'] = np.ascontiguousarray(-2.0 * embT[128:256])
    w['vq_e2'] = (emb.astype(np.float32) ** 2).sum(1)[None, :]  # [1, 64]
    w['vq_ones'] = np.ones((1, 64), np.float32)
    w['emb'] = np.ascontiguousarray(emb)
    w['identity'] = np.eye(128, dtype=np.float32)
    w['iota_rev'] = np.broadcast_to(
        (64.0 - np.arange(64, dtype=np.float32))[None, :], (128, 64)).copy()
    return w, emb


def _split_multi_waits(nc, mybir, limit=1):
    # walrus in this container rejects >1 sync wait per instruction
    # ("Too many sync wait commands"); hoist extras onto same-engine NoOps.
    f = nc.m.functions[0]
    for blk in f.blocks:
        new = []
        changed = False
        for i in blk.instructions:
            si = i.sync_info
            if si is not None and len(si.on_wait) > limit:
                waits = list(si.on_wait)
                for k, wt in enumerate(waits[:-limit]):
                    # real sem-wait instruction (EVSEM) -- a bare NoOp risks
                    # being dropped by codegen along with its waits
                    new.append(mybir.InstEventSemaphore(
                        name=f"{i.name}_wsplit{k}", engine=i.engine, ins=[], outs=[],
                        sync_info=mybir.SyncInfo(on_wait=[wt], on_update=[])))
                i.sync_info = mybir.SyncInfo(on_wait=waits[-limit:],
                                             on_update=list(si.on_update))
                changed = True
            new.append(i)
        if changed:
            blk.instructions = new


def _build_program():
    import concourse.bass as bass
    import concourse.tile as tile
    import concourse.mybir as mybir
    import contextlib
    dt = mybir.dt
    AF = mybir.ActivationFunctionType
    AL = mybir.AluOpType
    AX = mybir.AxisListType

    nc = bass.Bass("TRN2", target_bir_lowering=False, debug=False,
                   enable_asserts=False, num_devices=N_CORES)

    din = {}
    def inp(name, shape):
        din[name] = nc.dram_tensor(name, list(shape), dt.float32,
                                   kind="ExternalInput").ap()
        return din[name]

    mels = inp("mels", (B_SH, T, C))
    wsh = {
        'wn_tap': (80, 4, 3, 160), 'wn_rs': (80, 4, 160),
        'wn_in_bT': (80, 4, 2), 'wn_rs_bT': (80, 4, 2),
        'e1': (128, 10, 3, 160), 'e1d': (96, 10, 160), 'e_b1T': (80, 10, 2),
        'e2w': (80, 10, 2, 80), 'e_b2T': (80, 10),
        'post_tap': (80, 3, 256), 'post_bT': (128, 2),
        'vq_rhs1': (128, 64), 'vq_rhs2': (128, 64),
        'vq_e2': (1, 64), 'vq_ones': (1, 64),
        'emb': (64, 256), 'identity': (128, 128), 'iota_rev': (128, 64),
    }
    for k, s in wsh.items():
        inp(k, s)

    z_out = nc.dram_tensor("z_out", [B_SH * T, H], dt.float32,
                           kind="ExternalOutput").ap()
    dmin_out = nc.dram_tensor("dmin_out", [128, B_SH * NT * 4], dt.float32,
                              kind="ExternalOutput").ap()
    counts_out = nc.dram_tensor("counts_out", [1, B_SH * NT * 64], dt.float32,
                                kind="ExternalOutput").ap()

    with tile.TileContext(nc) as tc:
        est = contextlib.ExitStack()
        with est:
            wp = est.enter_context(tc.tile_pool(name="wp", bufs=1))
            rp = est.enter_context(tc.tile_pool(name="rp", bufs=1))
            wk = est.enter_context(tc.tile_pool(name="wk", bufs=2))

            # ---- load weights ----
            wt = {}
            for k, s in wsh.items():
                if k in ('e1', 'e1d', 'e2w'):
                    continue  # streamed per encoder block
                wt[k] = wp.tile(list(s), dt.float32, tag=k, name='w_'+k)
                nc.sync.dma_start(wt[k][:], din[k])
            ones128 = wp.tile([128, 1], dt.float32, tag="ones128")
            nc.vector.memset(ones128[:], 1.0)
            zeroT = wp.tile([128, 1], dt.float32, tag="zeroT")
            nc.vector.memset(zeroT[:], 0.0)
            epsT = wp.tile([128, 1], dt.float32, tag="epsT")
            nc.vector.memset(epsT[:], 1e-5)
            onesrow = wp.tile([1, TILE], dt.float32, tag="onesrow")
            nc.vector.memset(onesrow[:], 1.0)

            # ---- resident buffers [80, B_SH, TB] ----
            buf1 = rp.tile([C, B_SH, TB], dt.float32, tag="buf1")  # x / h / y
            buf2 = rp.tile([C, B_SH, TB], dt.float32, tag="buf2")  # out / xe
            buf3 = rp.tile([C, B_SH, TB], dt.float32, tag="buf3")  # h ping-pong
            for bf in (buf1, buf2, buf3):
                nc.vector.memset(bf[:, :, 0:PAD], 0.0)
                nc.vector.memset(bf[:, :, PAD + T:TB], 0.0)

            dmin_buf = rp.tile([128, B_SH * NT * 4], dt.float32, tag="dminb")
            nc.vector.memset(dmin_buf[:], 0.0)
            counts_buf = rp.tile([1, B_SH * NT * 64], dt.float32, tag="cntb")

            ident = wt['identity']

            # ---- phase 0: transpose input [T,80] -> x [80,T] ----
            with tc.tile_pool(name="ps0", bufs=4, space="PSUM") as ps0:
                for b in range(B_SH):
                    for cs in range(0, T, 128):
                        n = min(128, T - cs)
                        mt = wk.tile([128, C], dt.float32, tag="mt")
                        nc.sync.dma_start(mt[0:n, :], mels[b, cs:cs + n, :])
                        tp = ps0.tile([C, 128], dt.float32, tag="tp0")
                        nc.tensor.matmul(tp[0:C, 0:n], mt[0:n, 0:C],
                                         ident[0:n, 0:n], is_transpose=True)
                        nc.vector.tensor_copy(
                            buf1[:, b, PAD + cs:PAD + cs + n], tp[0:C, 0:n])

            # ---- phase 1: WaveNet (4 layers, per-tap convs) ----
            run_p1 = LIMIT_PHASES >= 1
            hbufs = [buf1, buf3, buf1, buf3]  # layer l reads hbufs[l]
            with tc.tile_pool(name="ps1", bufs=2, space="PSUM") as ps1:
                for l in range(4 if run_p1 else 0):
                    hsrc = hbufs[l]
                    hdst = buf3 if (l % 2 == 0) else buf1
                    for b in range(B_SH):
                        for t in range(NT):
                            t0 = PAD + t * TILE
                            psA = ps1.tile([C, TILE], dt.float32, tag="psA")
                            psB = ps1.tile([C, TILE], dt.float32, tag="psB")
                            for half, ps in ((0, psA), (1, psB)):
                                for k in range(3):
                                    rhs = hsrc[:, b, t0 - 1 + k:t0 - 1 + k + TILE]
                                    nc.tensor.matmul(
                                        ps[:], wt['wn_tap'][:, l, k, 80 * half:80 * half + 80],
                                        rhs, start=(k == 0), stop=(k == 2))
                            aA = wk.tile([C, TILE], dt.float32, tag="aA")
                            aB = wk.tile([C, TILE], dt.float32, tag="aB")
                            nc.scalar.activation(aA[:], psA[:], AF.Tanh,
                                                 bias=wt['wn_in_bT'][:, l, 0:1])
                            nc.scalar.activation(aB[:], psB[:], AF.Sigmoid,
                                                 bias=wt['wn_in_bT'][:, l, 1:2])
                            acts = wk.tile([C, TILE], dt.float32, tag="acts")
                            nc.vector.scalar_tensor_tensor(
                                acts[:], aA[:], 1.0, aB[:],
                                op0=AL.mult, op1=AL.mult)
                            psT = ps1.tile([C, TILE], dt.float32, tag="psTU", bufs=4)
                            nc.tensor.matmul(psT[:], wt['wn_rs'][:, l, 0:80],
                                             acts[:], start=True, stop=True)
                            dcol = hdst[:, b, t0:t0 + TILE]
                            ocol = buf2[:, b, t0:t0 + TILE]
                            if l < 3:
                                psU = ps1.tile([C, TILE], dt.float32, tag="psTU", bufs=4)
                                nc.tensor.matmul(psU[:], wt['wn_rs'][:, l, 80:160],
                                                 acts[:], start=True, stop=True)
                                nc.vector.scalar_tensor_tensor(
                                    dcol, psT[:], wt['wn_rs_bT'][:, l, 0:1],
                                    hsrc[:, b, t0:t0 + TILE],
                                    op0=AL.add, op1=AL.add)
                                if l == 0:
                                    nc.scalar.activation(
                                        ocol, psU[:], AF.Identity,
                                        bias=wt['wn_rs_bT'][:, l, 1:2])
                                else:
                                    nc.vector.scalar_tensor_tensor(
                                        ocol, psU[:], wt['wn_rs_bT'][:, l, 1:2],
                                        ocol, op0=AL.add, op1=AL.add)
                            else:
                                nc.vector.scalar_tensor_tensor(
                                    ocol, psT[:], wt['wn_rs_bT'][:, l, 0:1],
                                    ocol, op0=AL.add, op1=AL.add)

            # ---- layer-norm (transpose sandwich) ----
            def layer_norm(tpp, b, t0, xe, ybuf, ttag=("tpx", "tpy")):
                xeT = wk.tile([128, 4 * C], dt.float32, tag="xeT")
                for cch in range(4):
                    c0 = t0 + cch * PCH
                    tpx = tpp.tile([128, C], dt.float32, tag=ttag[0])
                    nc.tensor.matmul(tpx[0:PCH, 0:C], xe[:, b, c0:c0 + PCH],
                                     ident[0:C, 0:C], is_transpose=True)
                    nc.vector.tensor_copy(
                        xeT[0:PCH, cch * C:(cch + 1) * C], tpx[0:PCH, 0:C])
                xeT3 = xeT[0:PCH].rearrange("p (c f) -> p c f", c=4)
                ls = wk.tile([128, 20], dt.float32, tag="lnstat")
                mu, ssq, musq, var, rstd = (ls[:, 0:4], ls[:, 4:8], ls[:, 8:12],
                                            ls[:, 12:16], ls[:, 16:20])
                nc.vector.tensor_reduce(mu[0:PCH], xeT3, axis=AX.X, op=AL.add)
                sq = wk.tile([128, 4 * C], dt.float32, tag="sq")
                nc.scalar.activation(sq[0:PCH], xeT[0:PCH], AF.Square, bias=zeroT[0:PCH])
                nc.vector.tensor_reduce(
                    ssq[0:PCH], sq[0:PCH].rearrange("p (c f) -> p c f", c=4),
                    axis=AX.X, op=AL.add)
                nc.scalar.mul(mu[0:PCH], mu[0:PCH], 1.0 / C)
                nc.scalar.activation(musq[0:PCH], mu[0:PCH], AF.Square, bias=zeroT[0:PCH])
                nc.vector.scalar_tensor_tensor(
                    var[0:PCH], ssq[0:PCH], 1.0 / C, musq[0:PCH],
                    op0=AL.mult, op1=AL.subtract)
                nc.scalar.activation(var[0:PCH], var[0:PCH], AF.Sqrt, bias=epsT[0:PCH])
                nc.vector.reciprocal(rstd[0:PCH], var[0:PCH])
                for cch in range(4):
                    c0 = t0 + cch * PCH
                    yT = wk.tile([128, C], dt.float32, tag="yT")
                    nc.vector.tensor_scalar(
                        yT[0:PCH, :], xeT[0:PCH, cch * C:(cch + 1) * C],
                        mu[0:PCH, cch:cch + 1], rstd[0:PCH, cch:cch + 1],
                        op0=AL.subtract, op1=AL.mult)
                    tpy = tpp.tile([C, PCH], dt.float32, tag=ttag[1])
                    nc.tensor.matmul(tpy[0:C, 0:PCH], yT[0:PCH, 0:C],
                                     ident[0:PCH, 0:PCH], is_transpose=True)
                    nc.vector.tensor_copy(ybuf[:, b, c0:c0 + PCH], tpy[0:C, 0:PCH])

            # ---- phase 2: encoder blocks ----
            y = buf1
            xe = buf2
            memset_count = [0]
            with (tc.tile_pool(name="ps2t", bufs=1, space="PSUM") as ps2t,
                  tc.tile_pool(name="ps2c", bufs=2, space="PSUM") as ps2c,
                  tc.tile_pool(name="spool", bufs=2) as spool):
                for l in range(10 if LIMIT_PHASES >= 2 else 0):
                    e1s = spool.tile([128, 3, 160], dt.float32, tag="e1s",
                                     name=f"e1s_{l}")
                    nc.sync.dma_start(e1s[:], din['e1'][:, l])
                    e1ds = spool.tile([96, 160], dt.float32, tag="e1ds",
                                      name=f"e1ds_{l}")
                    nc.sync.dma_start(e1ds[:], din['e1d'][:, l])
                    e2ws = spool.tile([80, 2, 80], dt.float32, tag="e2ws",
                                      name=f"e2ws_{l}")
                    nc.sync.dma_start(e2ws[:], din['e2w'][:, l])
                    for b in range(B_SH):
                        for t in range(NT):
                            layer_norm(ps2t, b, PAD + t * TILE, xe, y)
                    for b in range(B_SH):
                        for t in range(NT):
                            t0 = PAD + t * TILE
                            ca = wk.tile([128, TILE], dt.float32, tag="ca")
                            cb = wk.tile([128, TILE], dt.float32, tag="cb")
                            cc = wk.tile([128, TILE], dt.float32, tag="cc")
                            cd = wk.tile([96, TILE], dt.float32, tag="cd")
                            if memset_count[0] < 3:  # zero pad rows on first slot uses
                                for cx in (ca, cb, cc, cd):
                                    nc.vector.memset(cx[:], 0.0)
                                memset_count[0] += 1
                            g = nc.gpsimd
                            # tap k lives at stack rows 96k..96k+80, reads col
                            # t0-2+k; pieces obey the <=32-span rule for
                            # non-zero base partitions
                            k0 = y[:, b, t0 - 2:t0 - 2 + TILE]
                            k1 = y[:, b, t0 - 1:t0 - 1 + TILE]
                            k2 = y[:, b, t0:t0 + TILE]
                            k3 = y[:, b, t0 + 1:t0 + 1 + TILE]
                            k4 = y[:, b, t0 + 2:t0 + 2 + TILE]
                            g.tensor_copy(ca[0:80, :], k0)
                            g.tensor_copy(ca[96:128, :], k1[0:32])
                            g.tensor_copy(cb[0:32, :], k1[32:64])
                            g.tensor_copy(cb[32:48, :], k1[64:80])
                            g.tensor_copy(cb[64:128, :], k2[0:64])
                            g.tensor_copy(cc[0:16, :], k2[64:80])
                            g.tensor_copy(cc[32:64, :], k3[0:32])
                            g.tensor_copy(cc[64:96, :], k3[32:64])
                            g.tensor_copy(cc[96:112, :], k3[64:80])
                            g.tensor_copy(cd[0:80, :], k4)
                            psA = ps2c.tile([C, TILE], dt.float32, tag="e_psA")
                            psB = ps2c.tile([C, TILE], dt.float32, tag="e_psB")
                            for half, ps in ((0, psA), (1, psB)):
                                h0 = 80 * half
                                for ci, ct in enumerate((ca, cb, cc)):
                                    nc.tensor.matmul(
                                        ps[:], e1s[:, ci, h0:h0 + 80],
                                        ct[:], start=(ci == 0), stop=False)
                                nc.tensor.matmul(
                                    ps[:], e1ds[:, h0:h0 + 80],
                                    cd[:], start=False, stop=True)
                            gA = wk.tile([C, TILE], dt.float32, tag="gA")
                            gB = wk.tile([C, TILE], dt.float32, tag="gB")
                            nc.scalar.activation(gA[:], psA[:], AF.Gelu,
                                                 bias=wt['e_b1T'][:, l, 0:1],
                                                 scale=SCALE5)
                            nc.scalar.activation(gB[:], psB[:], AF.Gelu,
                                                 bias=wt['e_b1T'][:, l, 1:2],
                                                 scale=SCALE5)
                            psC = ps2c.tile([C, TILE], dt.float32, tag="e_psC")
                            nc.tensor.matmul(psC[:], e2ws[:, 0, :], gA[:],
                                             start=True, stop=False)
                            nc.tensor.matmul(psC[:], e2ws[:, 1, :], gB[:],
                                             start=False, stop=True)
                            xcol = xe[:, b, t0:t0 + TILE]
                            nc.vector.scalar_tensor_tensor(
                                xcol, psC[:], wt['e_b2T'][:, l:l + 1], xcol,
                                op0=AL.add, op1=AL.add)

            # ---- phase 3: final LN + post conv (per-tap) + VQ ----
            with (tc.tile_pool(name="ps3t", bufs=2, space="PSUM") as ps3t,
                  tc.tile_pool(name="ps3", bufs=1, space="PSUM") as ps3):
                for b in range(B_SH if LIMIT_PHASES >= 3 else 0):
                    for t in range(NT):
                        layer_norm(ps3t, b, PAD + t * TILE, xe, y,
                                   ttag=("tp3", "tp3"))
                for b in range(B_SH if LIMIT_PHASES >= 3 else 0):
                    for t in range(NT):
                        t0 = PAD + t * TILE
                        tidx = b * NT + t
                        psP1 = ps3.tile([128, TILE], dt.float32, tag="psP1")
                        psP2 = ps3.tile([128, TILE], dt.float32, tag="psP2")
                        for ps, h0 in ((psP1, 0), (psP2, 128)):
                            for k in range(3):
                                rhs = y[:, b, t0 - 1 + k:t0 - 1 + k + TILE]
                                nc.tensor.matmul(
                                    ps[:], wt['post_tap'][:, k, h0:h0 + 128],
                                    rhs, start=(k == 0), stop=(k == 2))
                        pr1 = wk.tile([128, TILE], dt.float32, tag="pr1")
                        pr2 = wk.tile([128, TILE], dt.float32, tag="pr2")
                        nc.scalar.activation(pr1[:], psP1[:], AF.Identity,
                                             bias=wt['post_bT'][:, 0:1])
                        nc.scalar.activation(pr2[:], psP2[:], AF.Identity,
                                             bias=wt['post_bT'][:, 1:2])
                        sq1 = wk.tile([128, TILE], dt.float32, tag="sq1")
                        sq2 = wk.tile([128, TILE], dt.float32, tag="sq2")
                        nc.scalar.activation(sq1[:], pr1[:], AF.Square, bias=zeroT[:])
                        nc.scalar.activation(sq2[:], pr2[:], AF.Square, bias=zeroT[:])
                        psX = ps3.tile([1, TILE], dt.float32, tag="pXC", bufs=2)
                        nc.tensor.matmul(psX[:], ones128[:], sq1[:],
                                         start=True, stop=False)
                        nc.tensor.matmul(psX[:], ones128[:], sq2[:],
                                         start=False, stop=True)
                        x2row = wk.tile([1, TILE], dt.float32, tag="x2row")
                        nc.vector.tensor_copy(x2row[:], psX[:])
                        psCnt_t = ps3.tile([1, TILE], dt.float32, tag="pXC", bufs=2, name="psCnt")
                        psCnt = psCnt_t[:, 0:64]
                        for cch in range(4):
                            c0 = cch * PCH
                            psD_t = ps3.tile([PCH, H], dt.float32, tag="psDQ", bufs=2, name="psD")
                            psD = psD_t[:, 0:64]
                            nc.tensor.matmul(psD[:], pr1[:, c0:c0 + PCH],
                                             wt['vq_rhs1'][:], start=True, stop=False)
                            nc.tensor.matmul(psD[:], pr2[:, c0:c0 + PCH],
                                             wt['vq_rhs2'][:], start=False, stop=False)
                            nc.tensor.matmul(psD[:], onesrow[0:1, c0:c0 + PCH],
                                             wt['vq_e2'][:], start=False, stop=False)
                            nc.tensor.matmul(psD[:], x2row[0:1, c0:c0 + PCH],
                                             wt['vq_ones'][:], start=False, stop=True)
                            dmn = dmin_buf[0:PCH, tidx * 4 + cch:tidx * 4 + cch + 1]
                            nc.vector.tensor_reduce(dmn, psD[:], axis=AX.X, op=AL.min)
                            eq = wk.tile([PCH, 64], dt.float32, tag="eq")
                            nc.vector.scalar_tensor_tensor(
                                eq[:], psD[:], dmn, wt['iota_rev'][0:PCH, :],
                                op0=AL.is_equal, op1=AL.mult)
                            rev = wk.tile([PCH, 1], dt.float32, tag="rev")
                            nc.vector.tensor_reduce(rev[:], eq[:], axis=AX.X, op=AL.max)
                            oh = wk.tile([PCH, 64], dt.float32, tag="oh")
                            nc.vector.tensor_scalar(
                                oh[:], wt['iota_rev'][0:PCH, :], rev[:], None,
                                op0=AL.is_equal)
                            tpo = ps3t.tile([64, PCH], dt.float32, tag="tp3")
                            nc.tensor.matmul(tpo[0:64, 0:PCH], oh[0:PCH, 0:64],
                                             ident[0:PCH, 0:PCH], is_transpose=True)
                            ohT = wk.tile([64, PCH], dt.float32, tag="ohT")
                            nc.vector.tensor_copy(ohT[:], tpo[0:64, 0:PCH])
                            psQ = ps3.tile([PCH, H], dt.float32, tag="psDQ", bufs=2)
                            nc.tensor.matmul(psQ[:], ohT[0:64, 0:PCH],
                                             wt['emb'][:], start=True, stop=True)
                            zt = wk.tile([PCH, H], dt.float32, tag="zt")
                            nc.scalar.copy(zt[:], psQ[:])
                            p0 = b * T + t * TILE + cch * PCH
                            nc.sync.dma_start(z_out[p0:p0 + PCH, :], zt[:])
                            nc.tensor.matmul(psCnt[:], ones128[0:PCH, :], oh[:],
                                             start=(cch == 0), stop=(cch == 3),
                                             skip_group_check=True)
                        nc.vector.tensor_copy(
                            counts_buf[:, tidx * 64:(tidx + 1) * 64], psCnt[:])

                nc.sync.dma_start(dmin_out, dmin_buf[:])
                nc.sync.dma_start(counts_out, counts_buf[:])

    _split_multi_waits(nc, mybir)
    return nc


def _np_fallback(ref_mels, params):
    # Exact-semantics numpy fallback (only hit if inputs contain padding
    # frames, which randn inputs never do).
    from math import erf
    p = {k: ([_to_np(v) for v in params[k]] if isinstance(params[k], (list, tuple))
             else _to_np(params[k])) for k in params}
    x = ref_mels.transpose(0, 2, 1).astype(np.float32)
    mask = (ref_mels[:, :, 0] != 0.0).astype(np.float32)[:, None, :]

    def conv(xx, w, bias, pad):
        Bb, Cin, Tt = xx.shape
        Cout, _, K = w.shape
        xp = np.zeros((Bb, Cin, Tt + 2 * pad), np.float32)
        xp[:, :, pad:pad + Tt] = xx
        yy = np.zeros((Bb, Cout, Tt), np.float32)
        for k in range(K):
            yy += np.einsum('bct,oc->bot', xp[:, :, k:k + Tt], w[:, :, k])
        return yy + bias[None, :, None]

    out = np.zeros_like(x); h = x
    for i in range(4):
        xi = conv(h, p['wn_in_w'][i], p['wn_in_b'][i], 1)
        acts = np.tanh(xi[:, :C]) * (1.0 / (1.0 + np.exp(-xi[:, C:])))
        rs = conv(acts, p['wn_rs_w'][i], p['wn_rs_b'][i], 0)
        if i < 3:
            h = (h + rs[:, :C]) * mask
            out = out + rs[:, C:]
        else:
            out = out + rs
    xe = out * mask
    nonpad = (np.abs(xe).sum(1) > 0).astype(np.float32)[:, None, :]
    erfv = np.vectorize(erf)
    for i in range(10):
        mu = xe.mean(1, keepdims=True); var = xe.var(1, keepdims=True)
        hh = (xe - mu) / np.sqrt(var + 1e-5)
        hh = hh * p['enc_ln_w'][i][None, :, None] + p['enc_ln_b'][i][None, :, None]
        hh = conv(hh, p['enc_w1'][i], p['enc_b1'][i], 2) * SCALE5
        hh = hh * 0.5 * (1 + erfv(hh / np.sqrt(2.0)))
        hh = conv(hh, p['enc_w2'][i], p['enc_b2'][i], 0)
        xe = (xe + hh) * nonpad
    mu = xe.mean(1, keepdims=True); var = xe.var(1, keepdims=True)
    xe = (xe - mu) / np.sqrt(var + 1e-5)
    xe = (xe * p['last_ln_w'][None, :, None] + p['last_ln_b'][None, :, None]) * nonpad
    pros = (conv(xe, p['post_w'], p['post_b'], 1) * nonpad).transpose(0, 2, 1)
    emb = p['embedding']
    xf = pros.reshape(-1, H)
    dists = ((emb * emb).sum(1)[None, :] + (xf * xf).sum(1, keepdims=True)
             - 2.0 * (xf @ emb.T))
    idx = dists.argmin(-1)
    q = emb[idx].reshape(pros.shape)
    mse = ((pros - q) ** 2).mean(-1)
    np2 = (np.abs(pros).sum(-1) > 0).astype(np.float32)
    loss = np.float32(0.25 * (mse * np2).sum() / np2.sum())
    counts = np.bincount(idx, minlength=M).astype(np.float32)
    avg = counts / idx.shape[0]
    ppl = np.float32(np.exp(-np.sum(avg * np.log(avg + 1e-10))))
    return q.astype(np.float32), loss, ppl


def kernel(**inputs):
    ref_mels = _to_np(inputs['ref_mels'])           # [16, 4000, 80]
    params = inputs['params']
    if np.any(ref_mels[:, :, 0] == 0.0):
        return _np_fallback(ref_mels, params)

    from concourse.bass_utils import run_bass_kernel_spmd
    weights, emb = _prep_weights(params)
    if 'nc' not in _CACHE:
        _CACHE['nc'] = _build_program()
    nc = _CACHE['nc']

    in_maps = []
    for core in range(N_CORES):
        m = dict(weights)
        m['mels'] = np.ascontiguousarray(ref_mels[core * B_SH:(core + 1) * B_SH])
        in_maps.append(m)
    res = run_bass_kernel_spmd(nc, in_maps, core_ids=list(range(N_CORES)))

    z = np.empty((16, T, H), np.float32)
    dmin_total = 0.0
    counts = np.zeros(64, np.float64)
    for core in range(N_CORES):
        r = res.results[core]
        z[core * B_SH:(core + 1) * B_SH] = r['z_out'].reshape(B_SH, T, H)
        dmin_total += r['dmin_out'].astype(np.float64).sum()
        counts += r['counts_out'].reshape(-1, 64).astype(np.float64).sum(0)
    npos = 16 * T
    loss = np.float32(0.25 * dmin_total / (H * npos))
    avg = (counts / npos).astype(np.float32)
    ppl = np.float32(np.exp(-np.sum(avg * np.log(avg + 1e-10))))
    return z, loss, ppl
